# revision 4
# baseline (speedup 1.0000x reference)
"""Trainium2 Bass kernel for nn_CapsuleSubLayer (capsule routing).

Math (per head h):
  uh[b,d,j] = sum_s W[h,d,j,s] * x[h,b,s,d]            (batched matmul over d)
  3 routing iterations of softmax / weighted-sum / squash / logit update
  out[b,d,n,h] = v[h,b,d]  (broadcast over n)

Sharding: heads are fully independent -> 2 heads per NeuronCore on 8 cores.
Host-side we pre-permute x and W into DMA-friendly per-(h,d) layouts (and
cast to bf16; PSUM accumulation stays fp32):
  xt[h,d,p,c*64+b] = x[h,b,c*128+p,d]
  wt[h,d,p,c*16+n] = W[h,d,n,c*128+p]
so each per-(h,d) matmul chunk is lhsT=[p,b] (stationary), rhs=[p,n] (moving),
accumulated over c=0..7 into PSUM out[b,n].

Routing runs with partition=b (64 lanes) in fp32, in chunks of 32 d's so it
overlaps the next chunk's DMA/matmul: reductions over n are inner-free
reduces; the mean over b uses a ones-matmul on the PE which also leaves the
result replicated across partitions (exactly the layout the next softmax
needs).
"""

import os
import sys

import numpy as np

for _p in ("/opt/trn_rl_repo",):
    if _p not in sys.path:
        sys.path.insert(0, _p)

from contextlib import ExitStack

import ml_dtypes

import concourse.bass as bass
import concourse.tile as tile
from concourse import bacc, mybir
from concourse.bass_utils import run_bass_kernel_spmd

F32 = mybir.dt.float32
BF16 = mybir.dt.bfloat16

H, B, S, D, N = 16, 64, 1024, 64, 16
NCORES = 8
H_LOC = H // NCORES  # 2 heads per core
C = S // 128  # 8 contraction chunks

USE_FP32 = bool(int(os.environ.get("KERNEL_FP32", "0")))
IN_DT = F32 if USE_FP32 else BF16
IN_NP = np.float32 if USE_FP32 else ml_dtypes.bfloat16

_cache = {}


def _build(num_routing: int):
    nc = bacc.Bacc(
        "TRN2", target_bir_lowering=False, debug=False, num_devices=NCORES
    )
    xt = nc.dram_tensor("xt", [H_LOC, D, 128, C * B], IN_DT, kind="ExternalInput").ap()
    wt = nc.dram_tensor("wt", [H_LOC, D, 128, C * N], IN_DT, kind="ExternalInput").ap()
    ones = nc.dram_tensor("ones", [B, B], F32, kind="ExternalInput").ap()
    vout = nc.dram_tensor("vout", [B, H_LOC * D], F32, kind="ExternalOutput").ap()

    DG = 8  # d's per DMA batch
    RC = 32  # d's per routing chunk (= PSUM bank group)

    with ExitStack() as ctx:
        tc = ctx.enter_context(tile.TileContext(nc))
        xpool = ctx.enter_context(tc.tile_pool(name="xp", bufs=3))
        wpool = ctx.enter_context(tc.tile_pool(name="wp", bufs=3))
        pspool = ctx.enter_context(tc.tile_pool(name="ps", bufs=3, space="PSUM"))
        bppool = ctx.enter_context(tc.tile_pool(name="bp", bufs=2, space="PSUM"))
        uhpool = ctx.enter_context(tc.tile_pool(name="uh", bufs=3))
        rpool = ctx.enter_context(tc.tile_pool(name="rt", bufs=3))
        spool = ctx.enter_context(tc.tile_pool(name="sm", bufs=6))
        singles = ctx.enter_context(tc.tile_pool(name="sg", bufs=1))

        ones_sb = singles.tile([B, B], F32)
        nc.sync.dma_start(out=ones_sb, in_=ones)
        vout_sb = singles.tile([B, H_LOC * D], F32)

        def routing(uh, vout_slice):
            """3-iteration dynamic routing on a [B, RC, N] uh chunk."""
            bl = rpool.tile([B, RC, N], F32, tag="bl")
            for it in range(num_routing):
                if it == 0:
                    s_raw = spool.tile([B, RC, 1], F32, tag="sr")
                    nc.vector.reduce_sum(s_raw, uh, mybir.AxisListType.X)
                    scale = 1.0 / N
                else:
                    e = rpool.tile([B, RC, N], F32, tag="e")
                    nc.scalar.activation(e, bl, mybir.ActivationFunctionType.Exp)
                    esum = spool.tile([B, RC, 1], F32, tag="es")
                    nc.vector.reduce_sum(esum, e, mybir.AxisListType.X)
                    erec = spool.tile([B, RC, 1], F32, tag="er")
                    nc.vector.reciprocal(erec, esum)
                    cu = rpool.tile([B, RC, N], F32, tag="cu")
                    nc.vector.tensor_mul(cu, e, uh)
                    s_raw = spool.tile([B, RC, 1], F32, tag="sr")
                    csum = spool.tile([B, RC, 1], F32, tag="cs")
                    nc.vector.reduce_sum(csum, cu, mybir.AxisListType.X)
                    nc.vector.tensor_mul(s_raw, csum, erec)
                    scale = 1.0

                # squash: v = s*|s| / (1 + s^2)
                m = spool.tile([B, RC, 1], F32, tag="m")
                nc.scalar.activation(
                    m, s_raw, mybir.ActivationFunctionType.Abs, scale=scale
                )
                if scale != 1.0:
                    s_sc = spool.tile([B, RC, 1], F32, tag="ssc")
                    nc.scalar.mul(s_sc, s_raw, scale)
                else:
                    s_sc = s_raw
                msq = spool.tile([B, RC, 1], F32, tag="mq")
                nc.vector.tensor_mul(msq, m, m)
                den = spool.tile([B, RC, 1], F32, tag="dn")
                nc.vector.tensor_scalar_add(den, msq, 1.0)
                rec = spool.tile([B, RC, 1], F32, tag="rc")
                nc.vector.reciprocal(rec, den)
                t1 = spool.tile([B, RC, 1], F32, tag="t1")
                nc.vector.tensor_mul(t1, m, rec)
                v = spool.tile([B, RC, 1], F32, tag="v")
                nc.vector.tensor_mul(v, t1, s_sc)

                if it < num_routing - 1:
                    uv = rpool.tile([B, RC, N], F32, tag="uv")
                    nc.vector.tensor_mul(uv, uh, v.to_broadcast((B, RC, N)))
                    bp = bppool.tile([B, RC, N], F32, tag="bp")
                    nc.tensor.matmul(bp, ones_sb, uv, start=True, stop=True)
                    if it == 0:
                        nc.scalar.mul(bl, bp, float(N) / B)
                    else:
                        tmp = spool.tile([B, RC, N], F32, tag="tmp")
                        nc.scalar.mul(tmp, bp, float(N) / B)
                        nc.vector.tensor_add(bl, bl, tmp)
                else:
                    nc.vector.tensor_copy(out=vout_slice, in_=v[:, :, 0])

        for h in range(H_LOC):
            ps = None
            uh = None
            for dg in range(D // DG):
                x_t = xpool.tile([128, DG, C * B], IN_DT)
                nc.sync.dma_start(
                    out=x_t,
                    in_=xt[h, dg * DG : (dg + 1) * DG].rearrange("d p f -> p d f"),
                )
                w_t = wpool.tile([128, DG, C * N], IN_DT)
                nc.sync.dma_start(
                    out=w_t,
                    in_=wt[h, dg * DG : (dg + 1) * DG].rearrange("d p f -> p d f"),
                )
                for dl in range(DG):
                    d = dg * DG + dl
                    if d % RC == 0:
                        ps = pspool.tile([B, RC, N], F32)
                        uh = uhpool.tile([B, RC, N], F32)
                    for c in range(C):
                        nc.tensor.matmul(
                            ps[:, d % RC, :],
                            x_t[:, dl, c * B : (c + 1) * B],
                            w_t[:, dl, c * N : (c + 1) * N],
                            start=(c == 0),
                            stop=(c == C - 1),
                        )
                    if d % RC == RC - 1:
                        nc.vector.tensor_copy(out=uh, in_=ps)
                        r0 = h * D + (d - (RC - 1))
                        routing(uh, vout_sb[:, r0 : r0 + RC])

        nc.sync.dma_start(out=vout, in_=vout_sb)
    nc.finalize()
    return nc


def _prep_core(x, W, k):
    xs = x[2 * k : 2 * k + 2]  # [2, B, S, D]
    xt = np.ascontiguousarray(
        xs.reshape(H_LOC, B, C, 128, D).transpose(0, 4, 3, 2, 1).astype(IN_NP)
    ).reshape(H_LOC, D, 128, C * B)
    ws = W[2 * k : 2 * k + 2]  # [2, D, N, S]
    wt = np.ascontiguousarray(
        ws.reshape(H_LOC, D, N, C, 128).transpose(0, 1, 4, 3, 2).astype(IN_NP)
    ).reshape(H_LOC, D, 128, C * N)
    return xt, wt


def kernel(x, W, num_routing):
    x = np.asarray(x, dtype=np.float32)
    W = np.asarray(W, dtype=np.float32)
    nr = int(num_routing)
    if nr not in _cache:
        _cache[nr] = _build(nr)
    nc = _cache[nr]

    ones = np.ones((B, B), dtype=np.float32)
    in_maps = []
    for k in range(NCORES):
        xt, wt = _prep_core(x, W, k)
        in_maps.append({"xt": xt, "wt": wt, "ones": ones})

    kernel.last_in_maps = in_maps
    res = run_bass_kernel_spmd(
        nc,
        in_maps,
        core_ids=list(range(NCORES)),
        trace=bool(int(os.environ.get("KERNEL_TRACE", "0"))),
    )
    kernel.last_result = res

    v_full = np.empty((H, B, D), dtype=np.float32)
    for k in range(NCORES):
        r = res.results[k]["vout"]  # [B, H_LOC*D]
        v_full[2 * k] = r[:, 0:D]
        v_full[2 * k + 1] = r[:, D : 2 * D]
    out = np.broadcast_to(
        v_full.transpose(1, 2, 0)[:, :, None, :], (B, D, N, H)
    )
    return np.ascontiguousarray(out)


# revision 5
# speedup vs baseline: 1.3469x; 1.3469x over previous
"""Trainium2 Bass kernel for nn_CapsuleSubLayer (capsule routing).

Math (per head h):
  uh[b,d,j] = sum_s W[h,d,j,s] * x[h,b,s,d]            (batched matmul over d)
  3 routing iterations of softmax / weighted-sum / squash / logit update
  out[b,d,n,h] = v[h,b,d]  (broadcast over n)

Sharding: heads are fully independent -> 2 heads per NeuronCore on 8 cores.
Host-side we pre-permute x and W into DMA-friendly per-(h,d) layouts (and
cast to bf16; PSUM accumulation stays fp32):
  xt[h,d,p,c*64+b] = x[h,b,c*128+p,d]
  wt[h,d,p,c*16+n] = W[h,d,n,c*128+p]
so each per-(h,d) matmul chunk is lhsT=[p,b] (stationary), rhs=[p,n] (moving),
accumulated over c=0..7 into PSUM out[b,n].

Routing runs with partition=b (64 lanes) in fp32, in chunks of 32 d's so it
overlaps the next chunk's DMA/matmul: reductions over n are inner-free
reduces; the mean over b uses a ones-matmul on the PE which also leaves the
result replicated across partitions (exactly the layout the next softmax
needs).
"""

import os
import sys

import numpy as np

for _p in ("/opt/trn_rl_repo",):
    if _p not in sys.path:
        sys.path.insert(0, _p)

from contextlib import ExitStack

import ml_dtypes

import concourse.bass as bass
import concourse.tile as tile
from concourse import bacc, mybir
from concourse.bass_utils import run_bass_kernel_spmd

F32 = mybir.dt.float32
BF16 = mybir.dt.bfloat16

H, B, S, D, N = 16, 64, 1024, 64, 16
NCORES = 8
H_LOC = H // NCORES  # 2 heads per core
C = S // 128  # 8 contraction chunks

USE_FP32 = bool(int(os.environ.get("KERNEL_FP32", "0")))
IN_DT = F32 if USE_FP32 else mybir.dt.float16
IN_NP = np.float32 if USE_FP32 else np.float16

_cache = {}


def _build(num_routing: int):
    nc = bacc.Bacc(
        "TRN2", target_bir_lowering=False, debug=False, num_devices=NCORES
    )
    xt = nc.dram_tensor("xt", [H_LOC, D, 128, C * B], IN_DT, kind="ExternalInput").ap()
    wt = nc.dram_tensor("wt", [H_LOC, D, 128, C * N], IN_DT, kind="ExternalInput").ap()
    ones = nc.dram_tensor("ones", [B, B], F32, kind="ExternalInput").ap()
    vout = nc.dram_tensor("vout", [B, H_LOC * D], F32, kind="ExternalOutput").ap()

    DG = 8  # d's per DMA batch
    RC = 32  # d's per routing chunk (= PSUM bank group)

    with ExitStack() as ctx:
        tc = ctx.enter_context(tile.TileContext(nc))
        xpool = ctx.enter_context(tc.tile_pool(name="xp", bufs=3))
        wpool = ctx.enter_context(tc.tile_pool(name="wp", bufs=3))
        pspool = ctx.enter_context(tc.tile_pool(name="ps", bufs=3, space="PSUM"))
        bppool = ctx.enter_context(tc.tile_pool(name="bp", bufs=2, space="PSUM"))
        uhpool = ctx.enter_context(tc.tile_pool(name="uh", bufs=3))
        rpool = ctx.enter_context(tc.tile_pool(name="rt", bufs=3))
        spool = ctx.enter_context(tc.tile_pool(name="sm", bufs=6))
        singles = ctx.enter_context(tc.tile_pool(name="sg", bufs=1))

        ones_sb = singles.tile([B, B], F32)
        nc.sync.dma_start(out=ones_sb, in_=ones)
        vout_sb = singles.tile([B, H_LOC * D], F32)

        def routing(uh, vout_slice):
            """3-iteration dynamic routing on a [B, RC, N] uh chunk."""
            bl = rpool.tile([B, RC, N], F32, tag="bl")
            for it in range(num_routing):
                if it == 0:
                    s_raw = spool.tile([B, RC, 1], F32, tag="sr")
                    nc.vector.reduce_sum(s_raw, uh, mybir.AxisListType.X)
                    scale = 1.0 / N
                else:
                    e = rpool.tile([B, RC, N], F32, tag="e")
                    nc.scalar.activation(e, bl, mybir.ActivationFunctionType.Exp)
                    esum = spool.tile([B, RC, 1], F32, tag="es")
                    nc.vector.reduce_sum(esum, e, mybir.AxisListType.X)
                    erec = spool.tile([B, RC, 1], F32, tag="er")
                    nc.vector.reciprocal(erec, esum)
                    cu = rpool.tile([B, RC, N], F32, tag="cu")
                    nc.vector.tensor_mul(cu, e, uh)
                    s_raw = spool.tile([B, RC, 1], F32, tag="sr")
                    csum = spool.tile([B, RC, 1], F32, tag="cs")
                    nc.vector.reduce_sum(csum, cu, mybir.AxisListType.X)
                    nc.vector.tensor_mul(s_raw, csum, erec)
                    scale = 1.0

                # squash: v = s*|s| / (1 + s^2)
                m = spool.tile([B, RC, 1], F32, tag="m")
                nc.scalar.activation(
                    m, s_raw, mybir.ActivationFunctionType.Abs, scale=scale
                )
                if scale != 1.0:
                    s_sc = spool.tile([B, RC, 1], F32, tag="ssc")
                    nc.scalar.mul(s_sc, s_raw, scale)
                else:
                    s_sc = s_raw
                msq = spool.tile([B, RC, 1], F32, tag="mq")
                nc.vector.tensor_mul(msq, m, m)
                den = spool.tile([B, RC, 1], F32, tag="dn")
                nc.vector.tensor_scalar_add(den, msq, 1.0)
                rec = spool.tile([B, RC, 1], F32, tag="rc")
                nc.vector.reciprocal(rec, den)
                t1 = spool.tile([B, RC, 1], F32, tag="t1")
                nc.vector.tensor_mul(t1, m, rec)
                v = spool.tile([B, RC, 1], F32, tag="v")
                nc.vector.tensor_mul(v, t1, s_sc)

                if it < num_routing - 1:
                    uv = rpool.tile([B, RC, N], F32, tag="uv")
                    nc.vector.tensor_mul(uv, uh, v.to_broadcast((B, RC, N)))
                    bp = bppool.tile([B, RC, N], F32, tag="bp")
                    nc.tensor.matmul(bp, ones_sb, uv, start=True, stop=True)
                    if it == 0:
                        nc.scalar.mul(bl, bp, float(N) / B)
                    else:
                        tmp = spool.tile([B, RC, N], F32, tag="tmp")
                        nc.scalar.mul(tmp, bp, float(N) / B)
                        nc.vector.tensor_add(bl, bl, tmp)
                else:
                    nc.vector.tensor_copy(out=vout_slice, in_=v[:, :, 0])

        for h in range(H_LOC):
            ps = None
            uh = None
            for dg in range(D // DG):
                x_t = xpool.tile([128, DG, C * B], IN_DT)
                nc.sync.dma_start(
                    out=x_t,
                    in_=xt[h, dg * DG : (dg + 1) * DG].rearrange("d p f -> p d f"),
                )
                w_t = wpool.tile([128, DG, C * N], IN_DT)
                nc.sync.dma_start(
                    out=w_t,
                    in_=wt[h, dg * DG : (dg + 1) * DG].rearrange("d p f -> p d f"),
                )
                for dl in range(DG):
                    d = dg * DG + dl
                    if d % RC == 0:
                        ps = pspool.tile([B, RC, N], F32)
                        uh = uhpool.tile([B, RC, N], F32)
                    for c in range(C):
                        nc.tensor.matmul(
                            ps[:, d % RC, :],
                            x_t[:, dl, c * B : (c + 1) * B],
                            w_t[:, dl, c * N : (c + 1) * N],
                            start=(c == 0),
                            stop=(c == C - 1),
                        )
                    if d % RC == RC - 1:
                        nc.vector.tensor_copy(out=uh, in_=ps)
                        r0 = h * D + (d - (RC - 1))
                        routing(uh, vout_sb[:, r0 : r0 + RC])

        nc.sync.dma_start(out=vout, in_=vout_sb)
    nc.finalize()
    return nc


def _prep_core(x, W, k):
    xs = x[2 * k : 2 * k + 2]  # [2, B, S, D]
    xt = np.ascontiguousarray(
        xs.reshape(H_LOC, B, C, 128, D).transpose(0, 4, 3, 2, 1).astype(IN_NP)
    ).reshape(H_LOC, D, 128, C * B)
    ws = W[2 * k : 2 * k + 2]  # [2, D, N, S]
    wt = np.ascontiguousarray(
        ws.reshape(H_LOC, D, N, C, 128).transpose(0, 1, 4, 3, 2).astype(IN_NP)
    ).reshape(H_LOC, D, 128, C * N)
    return xt, wt


def kernel(x, W, num_routing):
    x = np.asarray(x, dtype=np.float32)
    W = np.asarray(W, dtype=np.float32)
    nr = int(num_routing)
    if nr not in _cache:
        _cache[nr] = _build(nr)
    nc = _cache[nr]

    ones = np.ones((B, B), dtype=np.float32)
    in_maps = []
    for k in range(NCORES):
        xt, wt = _prep_core(x, W, k)
        in_maps.append({"xt": xt, "wt": wt, "ones": ones})

    kernel.last_in_maps = in_maps
    res = run_bass_kernel_spmd(
        nc,
        in_maps,
        core_ids=list(range(NCORES)),
        trace=bool(int(os.environ.get("KERNEL_TRACE", "0"))),
    )
    kernel.last_result = res

    v_full = np.empty((H, B, D), dtype=np.float32)
    for k in range(NCORES):
        r = res.results[k]["vout"]  # [B, H_LOC*D]
        v_full[2 * k] = r[:, 0:D]
        v_full[2 * k + 1] = r[:, D : 2 * D]
    out = np.broadcast_to(
        v_full.transpose(1, 2, 0)[:, :, None, :], (B, D, N, H)
    )
    return np.ascontiguousarray(out)
